# revision 21
# baseline (speedup 1.0000x reference)
"""3-layer GCN (PyG GCNConv semantics) on 8 Trainium2 NeuronCores via Bass.

Sharding (per the hint): nodes sharded across 8 cores, edges partitioned by
destination node, [128,128] weights replicated, source features
halo-exchanged with an AllGather per layer.

On-chip layout is feature-major (x^T tiles [128 feat, nodes]). Per layer:
  A) h[tile] = Lrelu(x^T tile).T @ W   (PE, x^T tile stationary)
     -> own h shard, node-major, to DRAM.
  B) AllGather h shards -> full h table [NPAD, 128] in every core's DRAM.
  C) Per destination tile: dma_gather the (padded) source rows, then
     segment-sum via accumulating matmuls with host-precomputed
     one-hot*weight blocks S^T  (out^T[feat,dst] += M[msg,feat].T @
     S^T[msg,dst]) -> x^T_{l+1} tile + bias.

dma_gather indices are int16, so the h table is split in 4 quarters of
NPAD/4 < 32768 rows; each destination tile's messages are grouped by
source quarter with a fixed block count per (tile, quarter). A two-phase
host packing (nodes->tiles balancing in-degree, then tiles->core-pairs
balancing per-(dst-tile, quarter) counts) keeps every (tile, quarter)
within its fixed budget.
"""

import heapq

import numpy as np

import concourse.bacc as bacc
import concourse.bass as bass
import concourse.mybir as mybir
import concourse.tile as tile
from concourse.bass_utils import run_bass_kernel_spmd

N_CORES = 8
N_QUART = 4          # source quarters (pairs of cores)
D = 128
P = 128


class Cfg:
    def __init__(self, n_nodes, n_edges, tiles_per_core, qblocks,
                 chunk_tiles):
        self.n_nodes = n_nodes
        self.n_edges = n_edges
        self.tpc = tiles_per_core
        self.qb = qblocks                 # 128-blocks per (tile, quarter)
        self.gpt = N_QUART * qblocks      # gather blocks per dst tile
        self.cpt = self.gpt + 1           # + the self-loop block
        self.chunk = chunk_tiles
        self.shard = tiles_per_core * P
        self.npad = self.shard * N_CORES
        self.qrows = self.npad // N_QUART      # must fit int16
        self.nblk = self.tpc * self.cpt        # S^T blocks per core
        self.ngmsg = self.tpc * self.gpt * P   # gathered messages per core
        assert self.npad >= n_nodes
        assert self.qrows <= 32768
        assert self.shard <= 32768
        self.n_tiles = N_CORES * tiles_per_core


FULL = Cfg(n_nodes=100000, n_edges=640000, tiles_per_core=98,
           qblocks=2, chunk_tiles=4)

# ------------------------------------------------------------- host prep


def _pack_nodes_to_tiles(deg, cfg):
    """Nodes -> anonymous tiles (128 each), balancing total in-degree."""
    n_tiles = cfg.n_tiles
    order = np.argsort(-deg, kind="stable")
    heap = [(0, t) for t in range(n_tiles)]
    heapq.heapify(heap)
    counts = np.zeros(n_tiles, dtype=np.int64)
    node_tile = np.empty(cfg.n_nodes, dtype=np.int64)
    node_slot = np.empty(cfg.n_nodes, dtype=np.int64)
    for n in order:
        load, t = heapq.heappop(heap)
        node_tile[n] = t
        node_slot[n] = counts[t]
        counts[t] += 1
        if counts[t] < P:
            heapq.heappush(heap, (load + int(deg[n]), t))
    return node_tile, node_slot


def _try_pack_quarters(M, cfg, order, soft_margin):
    n_tiles = cfg.n_tiles
    per_q = n_tiles // N_QUART
    soft = cfg.qb * P - soft_margin
    R = np.zeros((N_QUART, n_tiles), dtype=np.float64)
    sizes = np.zeros(N_QUART, dtype=np.int64)
    tile_quarter = np.full(n_tiles, -1, dtype=np.int64)
    for s in order:
        row = M[s]
        best_q, best_pen = -1, None
        for q in range(N_QUART):
            if sizes[q] >= per_q:
                continue
            nr = R[q] + row
            over = np.maximum(nr - soft, 0)
            pen = (float((over * over).sum()), float(nr.max()),
                   float(sizes[q]))
            if best_pen is None or pen < best_pen:
                best_q, best_pen = q, pen
        R[best_q] += row
        sizes[best_q] += 1
        tile_quarter[s] = best_q
    return tile_quarter, R.max()


def _repair_quarters(M, tq, cfg, rng, iters=4000):
    """Local-search swap repair: drive per-(dst tile, quarter) counts
    under the hard cap by swapping tiles between quarters."""
    cap = cfg.qb * P
    n_tiles = cfg.n_tiles
    Mf = M.astype(np.float64)
    R = np.zeros((N_QUART, n_tiles), dtype=np.float64)
    for s in range(n_tiles):
        R[tq[s]] += Mf[s]
    members = [list(np.where(tq == q)[0]) for q in range(N_QUART)]

    def viol(r):
        o = np.maximum(r - cap, 0)
        return (o * o).sum(axis=-1)

    stall = 0
    for _ in range(iters):
        if (R <= cap).all():
            return tq, 0.0
        q1, d = np.unravel_index(np.argmax(R - cap), R.shape)
        mem1 = members[q1]
        contrib = M[mem1, d]
        ncand = 4 if stall < 50 else 8
        cand1 = [mem1[i] for i in np.argsort(-contrib)[:ncand]]
        base1 = float(viol(R[q1]))
        best = None
        for s1 in cand1:
            r1_wo = R[q1] - Mf[s1]
            for q2 in range(N_QUART):
                if q2 == q1:
                    continue
                mem2 = np.asarray(members[q2])
                base2 = float(viol(R[q2]))
                nr1 = r1_wo[None, :] + Mf[mem2]           # [m2, n]
                nr2 = (R[q2] + Mf[s1])[None, :] - Mf[mem2]
                delta = viol(nr1) + viol(nr2) - base1 - base2
                i = int(np.argmin(delta))
                if best is None or delta[i] < best[0]:
                    best = (float(delta[i]), s1, int(mem2[i]), q1, q2)
        if best is None or best[0] >= -1e-9:
            stall += 1
            if stall > 100:
                break
            # random perturbation swap to escape plateau
            q2 = int(rng.integers(0, N_QUART - 1))
            q2 = q2 if q2 < q1 else q2 + 1
            s1 = members[q1][int(rng.integers(len(members[q1])))]
            s2 = members[q2][int(rng.integers(len(members[q2])))]
            best = (0.0, s1, s2, q1, q2)
        else:
            stall = 0
        _, s1, s2, qa, qb_ = best
        R[qa] += Mf[s2] - Mf[s1]
        R[qb_] += Mf[s1] - Mf[s2]
        members[qa].remove(s1)
        members[qa].append(s2)
        members[qb_].remove(s2)
        members[qb_].append(s1)
        tq[s1], tq[s2] = qb_, qa
    if (R <= cap).all():
        return tq, 0.0
    return tq, float(max(viol(R[q]) for q in range(N_QUART)))


def _pack_tiles_to_quarters(M, cfg):
    """Assign anonymous tiles to quarters (cfg.n_tiles//4 each) keeping
    per-(dst tile, quarter) message counts <= qb*128. M[s,d] = messages
    from tile s to tile d. Returns tile_quarter[s]."""
    hard_cap = cfg.qb * P
    rng = np.random.default_rng(1234)
    attempts = [np.argsort(-M.sum(axis=1), kind="stable"),
                rng.permutation(cfg.n_tiles)]
    best_max = None
    for order in attempts:
        tq, rmax = _try_pack_quarters(M, cfg, order, 16)
        if rmax <= hard_cap:
            return tq
        tq, vmax = _repair_quarters(M, tq, cfg, rng)
        if vmax == 0:
            return tq
        if best_max is None or vmax < best_max:
            best_max = vmax
    raise RuntimeError(f"quarter packing failed: viol {best_max}")


def prepare(x, edge_index, cfg):
    n = cfg.n_nodes
    src = np.asarray(edge_index[0], dtype=np.int64)
    dst = np.asarray(edge_index[1], dtype=np.int64)
    # degree includes the self loop (reference appends them)
    deg = (np.bincount(dst, minlength=n) + 1).astype(np.float64)
    dinv = 1.0 / np.sqrt(deg)
    w = (dinv[src] * dinv[dst]).astype(np.float32)   # edge weights
    w_self = (dinv * dinv).astype(np.float32)        # self-loop weights

    node_tile, node_slot = _pack_nodes_to_tiles(deg, cfg)

    # tile-to-tile message counts (edges only; self loops bypass the
    # gather entirely)
    stile = node_tile[src]
    dtile = node_tile[dst]
    n_tiles = cfg.n_tiles
    M = np.zeros((n_tiles, n_tiles), dtype=np.int64)
    np.add.at(M, (stile, dtile), 1)

    tile_quarter = _pack_tiles_to_quarters(M, cfg)

    # order tiles: quarter-major; within a quarter split across its 2 cores
    # balancing per-core load (greedy on tile in-degree)
    tile_pos = np.empty(n_tiles, dtype=np.int64)   # global tile position
    tload = M.sum(axis=0)                          # in-msgs per tile
    for q in range(N_QUART):
        tiles_q = np.where(tile_quarter == q)[0]
        tiles_q = tiles_q[np.argsort(-tload[tiles_q], kind="stable")]
        loads = [(0, 0), (0, 1)]
        buckets = [[], []]
        for t in tiles_q:
            loads.sort()
            l, c = loads[0]
            buckets[c].append(t)
            loads[0] = (l + int(tload[t]), c)
        for ci in range(2):
            core = q * 2 + ci
            for li, t in enumerate(buckets[ci]):
                tile_pos[t] = core * cfg.tpc + li

    row_id = tile_pos[node_tile] * P + node_slot     # h-table row per node

    # ---- per-edge message placement
    drow_tile = tile_pos[node_tile[dst]]             # global dst tile pos
    dslot = node_slot[dst]
    srow = row_id[src]
    squart = srow // cfg.qrows

    # message key: (dst tile, src quarter); slot within segment by order
    seg = drow_tile * N_QUART + squart
    seg_order = np.argsort(seg, kind="stable")
    seg_s = seg[seg_order]
    seg_cnt = np.bincount(seg_s, minlength=n_tiles * N_QUART)
    seg_cap = cfg.qb * P
    if seg_cnt.max() > seg_cap:
        raise RuntimeError(f"segment overflow {seg_cnt.max()} > {seg_cap}")
    seg_start = np.concatenate([[0], np.cumsum(seg_cnt)])
    within = np.arange(len(seg_s)) - seg_start[seg_s]

    # padded message slot within (tile, quarter) segment of size qb*128
    pslot = seg_s * seg_cap + within
    m_srow = np.zeros(n_tiles * N_QUART * seg_cap, dtype=np.int64)
    m_w = np.zeros(n_tiles * N_QUART * seg_cap, dtype=np.float32)
    m_dslot = np.zeros(n_tiles * N_QUART * seg_cap, dtype=np.int64)
    m_srow[pslot] = srow[seg_order]
    m_w[pslot] = w[seg_order]
    m_dslot[pslot] = dslot[seg_order]
    # local (within-quarter) gather index; pads point at quarter row 0
    m_qloc = m_srow % cfg.qrows

    # reshape to [tile, quarter, qb, 128]
    m_qloc = m_qloc.reshape(n_tiles, N_QUART, cfg.qb, P)
    m_w = m_w.reshape(n_tiles, N_QUART, cfg.qb, P)
    m_dslot = m_dslot.reshape(n_tiles, N_QUART, cfg.qb, P)

    # x^T permuted + padded
    xTp = np.zeros((D, cfg.npad), dtype=np.float32)
    xTp[:, row_id] = np.asarray(x, dtype=np.float32).T

    # ---- per-core device arrays
    # chunk structure (mirrors the device program)
    chunks = []
    t0 = 0
    while t0 < cfg.tpc:
        chunks.append((t0, min(cfg.chunk, cfg.tpc - t0)))
        t0 += cfg.chunk

    # per-row self-loop weight (0 for dummy rows)
    wself_row = np.zeros(cfg.npad, dtype=np.float32)
    wself_row[row_id] = w_self

    per_core = []
    for k in range(N_CORES):
        # gather-block order (chunk -> quarter -> tile -> qb)
        gblocks = []                # (tilepos, q, qb)
        sblocks = []                # (chunk index ranges for self blocks)
        border = []                 # full S^T block order: gather + self
        for (t0, nt) in chunks:
            for q in range(N_QUART):
                for ti in range(nt):
                    for qb_i in range(cfg.qb):
                        gblocks.append((k * cfg.tpc + t0 + ti, q, qb_i))
            for ti in range(nt):
                sblocks.append(k * cfg.tpc + t0 + ti)
        gb = np.asarray(gblocks, dtype=np.int64)
        pos, qq, qb_i = gb[:, 0], gb[:, 1], gb[:, 2]
        blk_qloc = m_qloc[pos, qq, qb_i]           # [ngblk, 128]
        blk_w = m_w[pos, qq, qb_i]
        blk_dslot = m_dslot[pos, qq, qb_i]

        # idx16 [128, ngmsg/16]: gather streams are contiguous in
        # gather-block order; message i at (16r + i%16, i//16).
        midx = blk_qloc.reshape(-1)
        ncols = cfg.ngmsg // 16
        idx16 = np.empty((16, ncols), dtype=np.int16)
        ar = np.arange(cfg.ngmsg)
        idx16[ar % 16, ar // 16] = midx.astype(np.int16)
        idx16 = np.tile(idx16, (8, 1))             # replicate to 128 parts

        # S^T swizzled [128, nblk*128], in full per-chunk block order:
        # [gather blocks (q,ti,qb)..., self blocks (ti)...]
        ngblk = len(gblocks)
        sts = np.zeros((P, cfg.nblk * P), dtype=np.float32)
        # fill gather blocks
        full_gidx = []              # S^T block index of each gather block
        full_sidx = []              # S^T block index of each self block
        bi = 0
        gbi = 0
        sbi = 0
        for (t0, nt) in chunks:
            for _ in range(nt * cfg.gpt):
                full_gidx.append(bi)
                bi += 1
            for _ in range(nt):
                full_sidx.append(bi)
                bi += 1
        assert bi == cfg.nblk
        full_gidx = np.asarray(full_gidx)
        full_sidx = np.asarray(full_sidx)
        bcol = (full_gidx[:, None] * P + blk_dslot)    # [ngblk, 128]
        prow = np.arange(P)[None, :].repeat(ngblk, axis=0)
        sts[prow.ravel(), bcol.ravel()] = blk_w.ravel()
        # self blocks: diagonal of per-row self weights
        spos = np.asarray(sblocks, dtype=np.int64)
        srows = spos[:, None] * P + np.arange(P)[None, :]
        sw = wself_row[srows]                          # [tpc, 128]
        scol = full_sidx[:, None] * P + np.arange(P)[None, :]
        sts[prow[:len(spos)].ravel(), scol.ravel()] = sw.ravel()

        per_core.append({
            "xT": np.ascontiguousarray(
                xTp[:, k * cfg.shard:(k + 1) * cfg.shard]),
            "idx16": idx16,
            "sts": sts,
        })
    return per_core, row_id


# ------------------------------------------------------------ bass build

_FP = mybir.dt.float32


def build_program(cfg, ablate=(), repeats=1):
    """ablate: subset of {"noA","noB","noC","nogather","nost","nomm"}
    for phase-isolation timing experiments (results are wrong).
    repeats: run the whole 3-layer body N times (timing instrument)."""
    nc = bacc.Bacc("TRN2", target_bir_lowering=False, debug=False,
                   num_devices=N_CORES)
    xT_in = nc.declare_dram_parameter("xT", [D, cfg.shard], _FP,
                                      isOutput=False)
    idx_in = nc.declare_dram_parameter("idx16", [P, cfg.ngmsg // 16],
                                       mybir.dt.int16, isOutput=False)
    sts_in = nc.declare_dram_parameter("sts", [P, cfg.nblk * P], _FP,
                                       isOutput=False)
    w_ins = [nc.declare_dram_parameter(f"W{i}", [D, D], _FP, isOutput=False)
             for i in range(3)]
    b_ins = [nc.declare_dram_parameter(f"b{i}", [D, 1], _FP, isOutput=False)
             for i in range(3)]
    out_dram = nc.declare_dram_parameter("out", [D, cfg.shard], _FP,
                                         isOutput=True)

    h_shard = nc.dram_tensor("h_shard", [cfg.shard, D], _FP)
    h_table = nc.dram_tensor("h_table", [cfg.npad, D], _FP,
                             addr_space="Shared")

    groups = [list(range(N_CORES))]
    chunk_cols = cfg.chunk * cfg.cpt * P

    with tile.TileContext(nc, num_cores=N_CORES) as tc:
        with (
            tc.tile_pool(name="const", bufs=1) as cpool,
            tc.tile_pool(name="actp", bufs=3) as actp,
            tc.tile_pool(name="hps", bufs=4, space="PSUM") as hps,
            tc.tile_pool(name="hsb", bufs=3) as hsbp,
            tc.tile_pool(name="msgp", bufs=2) as msgp,
            tc.tile_pool(name="stp", bufs=2) as stp,
            tc.tile_pool(name="ops", bufs=4, space="PSUM") as ops,
            tc.tile_pool(name="osb", bufs=3) as osbp,
        ):
            xa = cpool.tile([D, cfg.shard], _FP, tag="xa")
            xb = cpool.tile([D, cfg.shard], _FP, tag="xb")
            idxt = cpool.tile([P, cfg.ngmsg // 16], mybir.dt.int16,
                              tag="idxt")
            wts = [cpool.tile([D, D], _FP, tag=f"w{i}", name=f"w{i}")
                   for i in range(3)]
            bts = [cpool.tile([D, 1], _FP, tag=f"b{i}", name=f"b{i}")
                   for i in range(3)]

            nc.sync.dma_start(out=xa[:], in_=xT_in[:])
            nc.sync.dma_start(out=idxt[:], in_=idx_in[:])
            for i in range(3):
                nc.sync.dma_start(out=wts[i][:], in_=w_ins[i][:])
                nc.sync.dma_start(out=bts[i][:], in_=b_ins[i][:])

            xbufs = [xa, xb]
            for layer3 in range(3 * repeats):
                layer = layer3 % 3
                xcur = xbufs[layer % 2]
                xnxt = xbufs[(layer + 1) % 2]
                with nc.named_scope(f"L{layer}_matmul"):
                    for t in range(cfg.tpc if "noA" not in ablate else 0):
                        cs = bass.ts(t, P)
                        act = actp.tile([D, P], _FP, tag="act")
                        nc.scalar.activation(
                            act[:], xcur[:, cs],
                            mybir.ActivationFunctionType.Lrelu, alpha=0.01)
                        hp = hps.tile([P, D], _FP, tag="hp")
                        nc.tensor.matmul(hp[:], lhsT=act[:],
                                         rhs=wts[layer][:],
                                         start=True, stop=True)
                        hs = hsbp.tile([P, D], _FP, tag="hs")
                        nc.vector.tensor_copy(out=hs[:], in_=hp[:])
                        nc.sync.dma_start(out=h_shard[t * P:(t + 1) * P, :],
                                          in_=hs[:])
                with nc.named_scope(f"L{layer}_allgather"):
                    if "noB" not in ablate:
                        nc.gpsimd.collective_compute(
                        "AllGather", mybir.AluOpType.bypass,
                            ins=[h_shard[:]], outs=[h_table[:]],
                            replica_groups=groups)
                with nc.named_scope(f"L{layer}_aggregate"):
                    blk0 = 0    # running S^T block offset
                    gmsg0 = 0   # running gathered-message offset
                    for t0 in range(0, cfg.tpc if "noC" not in ablate
                                    else 0, cfg.chunk):
                        nt = min(cfg.chunk, cfg.tpc - t0)
                        nb_chunk = nt * cfg.cpt
                        msg = msgp.tile([P, chunk_cols], _FP, tag="msg")
                        for q in range(N_QUART if "nogather" not in ablate
                                       else 0):
                            nb_q = nt * cfg.qb
                            nidx = nb_q * P
                            c0 = q * nb_q * P
                            i0 = (gmsg0 + q * nidx) // 16
                            nc.gpsimd.dma_gather(
                                out_ap=msg[:, c0:c0 + nidx].rearrange(
                                    "p (b e) -> p b e", e=P),
                                in_ap=h_table[q * cfg.qrows:
                                              (q + 1) * cfg.qrows, :],
                                idxs_ap=idxt[:, i0:i0 + nidx // 16],
                                num_idxs=nidx,
                                num_idxs_reg=nidx,
                                elem_size=P,
                            )
                        # self-loop rows: contiguous read of own h shard
                        sc0 = nt * cfg.gpt * P
                        nc.sync.dma_start(
                            out=msg[:, sc0:sc0 + nt * P].rearrange(
                                "p (b e) -> p b e", e=P),
                            in_=h_shard[t0 * P:(t0 + nt) * P, :].rearrange(
                                "(b p) e -> p b e", p=P))
                        st = stp.tile([P, chunk_cols], _FP, tag="st")
                        if "nost" not in ablate:
                            nc.sync.dma_start(
                                out=st[:, :nb_chunk * P],
                                in_=sts_in[:, blk0 * P:(blk0 + nb_chunk) * P])
                        for ti in range(nt if "nomm" not in ablate else 0):
                            t = t0 + ti
                            op = ops.tile([D, P], _FP, tag="op")
                            ci = 0
                            last = cfg.cpt - 1
                            for q in range(N_QUART):
                                for qb_i in range(cfg.qb):
                                    col = ((q * nt + ti) * cfg.qb
                                           + qb_i) * P
                                    nc.tensor.matmul(
                                        op[:], lhsT=msg[:, col:col + P],
                                        rhs=st[:, col:col + P],
                                        start=(ci == 0),
                                        stop=(ci == last))
                                    ci += 1
                            # self block
                            mcol = sc0 + ti * P
                            scol = (nt * cfg.gpt + ti) * P
                            nc.tensor.matmul(
                                op[:], lhsT=msg[:, mcol:mcol + P],
                                rhs=st[:, scol:scol + P],
                                start=False, stop=True)
                            cs = bass.ts(t, P)
                            if layer < 2:
                                nc.vector.tensor_scalar_add(
                                    out=xnxt[:, cs], in0=op[:],
                                    scalar1=bts[layer][:])
                            else:
                                ob = osbp.tile([D, P], _FP, tag="ob")
                                nc.vector.tensor_scalar_add(
                                    out=ob[:], in0=op[:],
                                    scalar1=bts[layer][:])
                                nc.sync.dma_start(out=out_dram[:, cs],
                                                  in_=ob[:])
                        blk0 += nb_chunk
                        gmsg0 += nt * cfg.gpt * P
    nc.compile()
    return nc


_PROGRAM_CACHE = {}


def _get_program(cfg):
    key = (cfg.n_nodes, cfg.n_edges, cfg.tpc, cfg.qb, cfg.chunk)
    if key not in _PROGRAM_CACHE:
        _PROGRAM_CACHE[key] = build_program(cfg)
    return _PROGRAM_CACHE[key]


# --------------------------------------------------------------- driver


def run(x, edge_index, W1, b1, W2, b2, W3, b3, cfg, trace=False,
        trace_kwargs=None):
    per_core, row_id = prepare(x, edge_index, cfg)
    nc = _get_program(cfg)
    ws = [np.asarray(a, dtype=np.float32) for a in (W1, W2, W3)]
    bs = [np.asarray(a, dtype=np.float32).reshape(D, 1) for a in (b1, b2, b3)]
    in_maps = []
    for k in range(N_CORES):
        m = dict(per_core[k])
        for i in range(3):
            m[f"W{i}"] = ws[i]
            m[f"b{i}"] = bs[i]
        in_maps.append(m)
    res = run_bass_kernel_spmd(nc, in_maps, list(range(N_CORES)),
                               trace=trace, **(trace_kwargs or {}))
    outT = np.concatenate([res.results[k]["out"] for k in range(N_CORES)],
                          axis=1)
    out = np.empty((cfg.n_nodes, D), dtype=np.float32)
    out[:, :] = outT[:, row_id].T
    return out, res


def kernel(x, edge_index, W1, b1, W2, b2, W3, b3):
    out, _ = run(x, edge_index, W1, b1, W2, b2, W3, b3, FULL)
    return out


# revision 23
# speedup vs baseline: 1.2220x; 1.2220x over previous
"""3-layer GCN (PyG GCNConv semantics) on 8 Trainium2 NeuronCores via Bass.

Sharding (per the hint): nodes sharded across 8 cores, edges partitioned by
destination node, [128,128] weights replicated, source features
halo-exchanged with an AllGather per layer.

On-chip layout is feature-major (x^T tiles [128 feat, nodes]). Per layer:
  A) h[tile] = Lrelu(x^T tile).T @ W   (PE, x^T tile stationary)
     -> own h shard, node-major, to DRAM.
  B) AllGather h shards -> full h table [NPAD, 128] in every core's DRAM.
  C) Per destination tile: dma_gather the (padded) source rows, then
     segment-sum via accumulating matmuls with host-precomputed
     one-hot*weight blocks S^T  (out^T[feat,dst] += M[msg,feat].T @
     S^T[msg,dst]) -> x^T_{l+1} tile + bias.

dma_gather indices are int16, so the h table is split in 4 quarters of
NPAD/4 < 32768 rows; each destination tile's messages are grouped by
source quarter with a fixed block count per (tile, quarter). A two-phase
host packing (nodes->tiles balancing in-degree, then tiles->core-pairs
balancing per-(dst-tile, quarter) counts) keeps every (tile, quarter)
within its fixed budget.
"""

import heapq

import numpy as np

import concourse.bacc as bacc
import concourse.bass as bass
import concourse.mybir as mybir
import concourse.tile as tile
from concourse.bass_utils import run_bass_kernel_spmd

N_CORES = 8
N_QUART = 4          # source quarters (pairs of cores)
D = 128
P = 128


class Cfg:
    def __init__(self, n_nodes, n_edges, tiles_per_core, qblocks,
                 chunk_tiles, fp16=True):
        self.n_nodes = n_nodes
        self.n_edges = n_edges
        self.tpc = tiles_per_core
        self.qb = qblocks                 # 128-blocks per (tile, quarter)
        self.gpt = N_QUART * qblocks      # gather blocks per dst tile
        self.cpt = self.gpt + 1           # + the self-loop block
        self.chunk = chunk_tiles
        self.shard = tiles_per_core * P
        self.npad = self.shard * N_CORES
        self.qrows = self.npad // N_QUART      # must fit int16
        self.nblk = self.tpc * self.cpt        # S^T blocks per core
        self.ngmsg = self.tpc * self.gpt * P   # gathered messages per core
        assert self.npad >= n_nodes
        assert self.qrows <= 32768
        assert self.shard <= 32768
        self.n_tiles = N_CORES * tiles_per_core
        self.fp16 = fp16


FULL = Cfg(n_nodes=100000, n_edges=640000, tiles_per_core=98,
           qblocks=2, chunk_tiles=4)

# ------------------------------------------------------------- host prep


def _pack_nodes_to_tiles(deg, cfg):
    """Nodes -> anonymous tiles (128 each), balancing total in-degree."""
    n_tiles = cfg.n_tiles
    order = np.argsort(-deg, kind="stable")
    heap = [(0, t) for t in range(n_tiles)]
    heapq.heapify(heap)
    counts = np.zeros(n_tiles, dtype=np.int64)
    node_tile = np.empty(cfg.n_nodes, dtype=np.int64)
    node_slot = np.empty(cfg.n_nodes, dtype=np.int64)
    for n in order:
        load, t = heapq.heappop(heap)
        node_tile[n] = t
        node_slot[n] = counts[t]
        counts[t] += 1
        if counts[t] < P:
            heapq.heappush(heap, (load + int(deg[n]), t))
    return node_tile, node_slot


def _try_pack_quarters(M, cfg, order, soft_margin):
    n_tiles = cfg.n_tiles
    per_q = n_tiles // N_QUART
    soft = cfg.qb * P - soft_margin
    R = np.zeros((N_QUART, n_tiles), dtype=np.float64)
    sizes = np.zeros(N_QUART, dtype=np.int64)
    tile_quarter = np.full(n_tiles, -1, dtype=np.int64)
    for s in order:
        row = M[s]
        best_q, best_pen = -1, None
        for q in range(N_QUART):
            if sizes[q] >= per_q:
                continue
            nr = R[q] + row
            over = np.maximum(nr - soft, 0)
            pen = (float((over * over).sum()), float(nr.max()),
                   float(sizes[q]))
            if best_pen is None or pen < best_pen:
                best_q, best_pen = q, pen
        R[best_q] += row
        sizes[best_q] += 1
        tile_quarter[s] = best_q
    return tile_quarter, R.max()


def _repair_quarters(M, tq, cfg, rng, iters=4000):
    """Local-search swap repair: drive per-(dst tile, quarter) counts
    under the hard cap by swapping tiles between quarters."""
    cap = cfg.qb * P
    n_tiles = cfg.n_tiles
    Mf = M.astype(np.float64)
    R = np.zeros((N_QUART, n_tiles), dtype=np.float64)
    for s in range(n_tiles):
        R[tq[s]] += Mf[s]
    members = [list(np.where(tq == q)[0]) for q in range(N_QUART)]

    def viol(r):
        o = np.maximum(r - cap, 0)
        return (o * o).sum(axis=-1)

    stall = 0
    for _ in range(iters):
        if (R <= cap).all():
            return tq, 0.0
        q1, d = np.unravel_index(np.argmax(R - cap), R.shape)
        mem1 = members[q1]
        contrib = M[mem1, d]
        ncand = 4 if stall < 50 else 8
        cand1 = [mem1[i] for i in np.argsort(-contrib)[:ncand]]
        base1 = float(viol(R[q1]))
        best = None
        for s1 in cand1:
            r1_wo = R[q1] - Mf[s1]
            for q2 in range(N_QUART):
                if q2 == q1:
                    continue
                mem2 = np.asarray(members[q2])
                base2 = float(viol(R[q2]))
                nr1 = r1_wo[None, :] + Mf[mem2]           # [m2, n]
                nr2 = (R[q2] + Mf[s1])[None, :] - Mf[mem2]
                delta = viol(nr1) + viol(nr2) - base1 - base2
                i = int(np.argmin(delta))
                if best is None or delta[i] < best[0]:
                    best = (float(delta[i]), s1, int(mem2[i]), q1, q2)
        if best is None or best[0] >= -1e-9:
            stall += 1
            if stall > 100:
                break
            # random perturbation swap to escape plateau
            q2 = int(rng.integers(0, N_QUART - 1))
            q2 = q2 if q2 < q1 else q2 + 1
            s1 = members[q1][int(rng.integers(len(members[q1])))]
            s2 = members[q2][int(rng.integers(len(members[q2])))]
            best = (0.0, s1, s2, q1, q2)
        else:
            stall = 0
        _, s1, s2, qa, qb_ = best
        R[qa] += Mf[s2] - Mf[s1]
        R[qb_] += Mf[s1] - Mf[s2]
        members[qa].remove(s1)
        members[qa].append(s2)
        members[qb_].remove(s2)
        members[qb_].append(s1)
        tq[s1], tq[s2] = qb_, qa
    if (R <= cap).all():
        return tq, 0.0
    return tq, float(max(viol(R[q]) for q in range(N_QUART)))


def _pack_tiles_to_quarters(M, cfg):
    """Assign anonymous tiles to quarters (cfg.n_tiles//4 each) keeping
    per-(dst tile, quarter) message counts <= qb*128. M[s,d] = messages
    from tile s to tile d. Returns tile_quarter[s]."""
    hard_cap = cfg.qb * P
    rng = np.random.default_rng(1234)
    attempts = [np.argsort(-M.sum(axis=1), kind="stable"),
                rng.permutation(cfg.n_tiles)]
    best_max = None
    for order in attempts:
        tq, rmax = _try_pack_quarters(M, cfg, order, 16)
        if rmax <= hard_cap:
            return tq
        tq, vmax = _repair_quarters(M, tq, cfg, rng)
        if vmax == 0:
            return tq
        if best_max is None or vmax < best_max:
            best_max = vmax
    raise RuntimeError(f"quarter packing failed: viol {best_max}")


def prepare(x, edge_index, cfg):
    n = cfg.n_nodes
    src = np.asarray(edge_index[0], dtype=np.int64)
    dst = np.asarray(edge_index[1], dtype=np.int64)
    # degree includes the self loop (reference appends them)
    deg = (np.bincount(dst, minlength=n) + 1).astype(np.float64)
    dinv = 1.0 / np.sqrt(deg)
    w = (dinv[src] * dinv[dst]).astype(np.float32)   # edge weights
    w_self = (dinv * dinv).astype(np.float32)        # self-loop weights

    node_tile, node_slot = _pack_nodes_to_tiles(deg, cfg)

    # tile-to-tile message counts (edges only; self loops bypass the
    # gather entirely)
    stile = node_tile[src]
    dtile = node_tile[dst]
    n_tiles = cfg.n_tiles
    M = np.zeros((n_tiles, n_tiles), dtype=np.int64)
    np.add.at(M, (stile, dtile), 1)

    tile_quarter = _pack_tiles_to_quarters(M, cfg)

    # order tiles: quarter-major; within a quarter split across its 2 cores
    # balancing per-core load (greedy on tile in-degree)
    tile_pos = np.empty(n_tiles, dtype=np.int64)   # global tile position
    tload = M.sum(axis=0)                          # in-msgs per tile
    for q in range(N_QUART):
        tiles_q = np.where(tile_quarter == q)[0]
        tiles_q = tiles_q[np.argsort(-tload[tiles_q], kind="stable")]
        loads = [(0, 0), (0, 1)]
        buckets = [[], []]
        for t in tiles_q:
            loads.sort()
            l, c = loads[0]
            buckets[c].append(t)
            loads[0] = (l + int(tload[t]), c)
        for ci in range(2):
            core = q * 2 + ci
            for li, t in enumerate(buckets[ci]):
                tile_pos[t] = core * cfg.tpc + li

    row_id = tile_pos[node_tile] * P + node_slot     # h-table row per node

    # ---- per-edge message placement
    drow_tile = tile_pos[node_tile[dst]]             # global dst tile pos
    dslot = node_slot[dst]
    srow = row_id[src]
    squart = srow // cfg.qrows

    # message key: (dst tile, src quarter); slot within segment by order
    seg = drow_tile * N_QUART + squart
    seg_order = np.argsort(seg, kind="stable")
    seg_s = seg[seg_order]
    seg_cnt = np.bincount(seg_s, minlength=n_tiles * N_QUART)
    seg_cap = cfg.qb * P
    if seg_cnt.max() > seg_cap:
        raise RuntimeError(f"segment overflow {seg_cnt.max()} > {seg_cap}")
    seg_start = np.concatenate([[0], np.cumsum(seg_cnt)])
    within = np.arange(len(seg_s)) - seg_start[seg_s]

    # padded message slot within (tile, quarter) segment of size qb*128
    pslot = seg_s * seg_cap + within
    m_srow = np.zeros(n_tiles * N_QUART * seg_cap, dtype=np.int64)
    m_w = np.zeros(n_tiles * N_QUART * seg_cap, dtype=np.float32)
    m_dslot = np.zeros(n_tiles * N_QUART * seg_cap, dtype=np.int64)
    m_srow[pslot] = srow[seg_order]
    m_w[pslot] = w[seg_order]
    m_dslot[pslot] = dslot[seg_order]
    # local (within-quarter) gather index; pads point at quarter row 0
    m_qloc = m_srow % cfg.qrows

    # reshape to [tile, quarter, qb, 128]
    m_qloc = m_qloc.reshape(n_tiles, N_QUART, cfg.qb, P)
    m_w = m_w.reshape(n_tiles, N_QUART, cfg.qb, P)
    m_dslot = m_dslot.reshape(n_tiles, N_QUART, cfg.qb, P)

    # x^T permuted + padded
    xTp = np.zeros((D, cfg.npad), dtype=np.float32)
    xTp[:, row_id] = np.asarray(x, dtype=np.float32).T

    # ---- per-core device arrays
    # chunk structure (mirrors the device program)
    chunks = []
    t0 = 0
    while t0 < cfg.tpc:
        chunks.append((t0, min(cfg.chunk, cfg.tpc - t0)))
        t0 += cfg.chunk

    # per-row self-loop weight (0 for dummy rows)
    wself_row = np.zeros(cfg.npad, dtype=np.float32)
    wself_row[row_id] = w_self

    per_core = []
    for k in range(N_CORES):
        # gather-block order (chunk -> quarter -> tile -> qb)
        gblocks = []                # (tilepos, q, qb)
        sblocks = []                # (chunk index ranges for self blocks)
        border = []                 # full S^T block order: gather + self
        for (t0, nt) in chunks:
            for q in range(N_QUART):
                for ti in range(nt):
                    for qb_i in range(cfg.qb):
                        gblocks.append((k * cfg.tpc + t0 + ti, q, qb_i))
            for ti in range(nt):
                sblocks.append(k * cfg.tpc + t0 + ti)
        gb = np.asarray(gblocks, dtype=np.int64)
        pos, qq, qb_i = gb[:, 0], gb[:, 1], gb[:, 2]
        blk_qloc = m_qloc[pos, qq, qb_i]           # [ngblk, 128]
        blk_w = m_w[pos, qq, qb_i]
        blk_dslot = m_dslot[pos, qq, qb_i]

        # idx16 [128, ngmsg/16]: gather streams are contiguous in
        # gather-block order; message i at (16r + i%16, i//16).
        midx = blk_qloc.reshape(-1)
        ncols = cfg.ngmsg // 16
        idx16 = np.empty((16, ncols), dtype=np.int16)
        ar = np.arange(cfg.ngmsg)
        idx16[ar % 16, ar // 16] = midx.astype(np.int16)
        idx16 = np.tile(idx16, (8, 1))             # replicate to 128 parts

        # S^T swizzled [128, nblk*128], in full per-chunk block order:
        # [gather blocks (q,ti,qb)..., self blocks (ti)...]
        ngblk = len(gblocks)
        sts = np.zeros((P, cfg.nblk * P), dtype=np.float32)
        # fill gather blocks
        full_gidx = []              # S^T block index of each gather block
        full_sidx = []              # S^T block index of each self block
        bi = 0
        gbi = 0
        sbi = 0
        for (t0, nt) in chunks:
            for _ in range(nt * cfg.gpt):
                full_gidx.append(bi)
                bi += 1
            for _ in range(nt):
                full_sidx.append(bi)
                bi += 1
        assert bi == cfg.nblk
        full_gidx = np.asarray(full_gidx)
        full_sidx = np.asarray(full_sidx)
        bcol = (full_gidx[:, None] * P + blk_dslot)    # [ngblk, 128]
        prow = np.arange(P)[None, :].repeat(ngblk, axis=0)
        sts[prow.ravel(), bcol.ravel()] = blk_w.ravel()
        # self blocks: diagonal of per-row self weights
        spos = np.asarray(sblocks, dtype=np.int64)
        srows = spos[:, None] * P + np.arange(P)[None, :]
        sw = wself_row[srows]                          # [tpc, 128]
        scol = full_sidx[:, None] * P + np.arange(P)[None, :]
        sts[prow[:len(spos)].ravel(), scol.ravel()] = sw.ravel()

        per_core.append({
            "xT": np.ascontiguousarray(
                xTp[:, k * cfg.shard:(k + 1) * cfg.shard]),
            "idx16": idx16,
            "sts": sts.astype(np.float16) if cfg.fp16 else sts,
        })
    return per_core, row_id


# ------------------------------------------------------------ bass build

_FP = mybir.dt.float32


def build_program(cfg, ablate=(), repeats=1):
    """ablate: subset of {"noA","noB","noC","nogather","nost","nomm"}
    for phase-isolation timing experiments (results are wrong).
    repeats: run the whole 3-layer body N times (timing instrument)."""
    nc = bacc.Bacc("TRN2", target_bir_lowering=False, debug=False,
                   num_devices=N_CORES)
    _HD = mybir.dt.float16 if cfg.fp16 else _FP
    xT_in = nc.declare_dram_parameter("xT", [D, cfg.shard], _FP,
                                      isOutput=False)
    idx_in = nc.declare_dram_parameter("idx16", [P, cfg.ngmsg // 16],
                                       mybir.dt.int16, isOutput=False)
    sts_in = nc.declare_dram_parameter("sts", [P, cfg.nblk * P], _HD,
                                       isOutput=False)
    w_ins = [nc.declare_dram_parameter(f"W{i}", [D, D], _FP, isOutput=False)
             for i in range(3)]
    b_ins = [nc.declare_dram_parameter(f"b{i}", [D, 1], _FP, isOutput=False)
             for i in range(3)]
    out_dram = nc.declare_dram_parameter("out", [D, cfg.shard], _FP,
                                         isOutput=True)

    h_shard = nc.dram_tensor("h_shard", [cfg.shard, D], _HD)
    h_table = nc.dram_tensor("h_table", [cfg.npad, D], _HD,
                             addr_space="Shared")

    groups = [list(range(N_CORES))]
    chunk_cols = cfg.chunk * cfg.cpt * P

    with tile.TileContext(nc, num_cores=N_CORES) as tc:
        with (
            tc.tile_pool(name="const", bufs=1) as cpool,
            tc.tile_pool(name="actp", bufs=3) as actp,
            tc.tile_pool(name="hps", bufs=4, space="PSUM") as hps,
            tc.tile_pool(name="hsb", bufs=3) as hsbp,
            tc.tile_pool(name="msgp", bufs=2) as msgp,
            tc.tile_pool(name="stp", bufs=2) as stp,
            tc.tile_pool(name="ops", bufs=4, space="PSUM") as ops,
            tc.tile_pool(name="osb", bufs=3) as osbp,
        ):
            xa = cpool.tile([D, cfg.shard], _FP, tag="xa")
            xb = cpool.tile([D, cfg.shard], _FP, tag="xb")
            idxt = cpool.tile([P, cfg.ngmsg // 16], mybir.dt.int16,
                              tag="idxt")
            wts = [cpool.tile([D, D], _FP, tag=f"w{i}", name=f"w{i}")
                   for i in range(3)]
            bts = [cpool.tile([D, 1], _FP, tag=f"b{i}", name=f"b{i}")
                   for i in range(3)]

            nc.sync.dma_start(out=xa[:], in_=xT_in[:])
            if "noC" in ablate or "nomm" in ablate:
                nc.gpsimd.memset(xb[:], 0.0)   # keep xb written (ablation)
            nc.sync.dma_start(out=idxt[:], in_=idx_in[:])
            for i in range(3):
                nc.sync.dma_start(out=wts[i][:], in_=w_ins[i][:])
                nc.sync.dma_start(out=bts[i][:], in_=b_ins[i][:])

            xbufs = [xa, xb]
            for layer3 in range(3 * repeats):
                layer = layer3 % 3
                xcur = xbufs[layer % 2]
                xnxt = xbufs[(layer + 1) % 2]
                with nc.named_scope(f"L{layer}_matmul"):
                    for t in range(cfg.tpc if "noA" not in ablate else 0):
                        cs = bass.ts(t, P)
                        act = actp.tile([D, P], _FP, tag="act")
                        nc.scalar.activation(
                            act[:], xcur[:, cs],
                            mybir.ActivationFunctionType.Lrelu, alpha=0.01)
                        hp = hps.tile([P, D], _FP, tag="hp")
                        nc.tensor.matmul(hp[:], lhsT=act[:],
                                         rhs=wts[layer][:],
                                         start=True, stop=True)
                        hs = hsbp.tile([P, D], _HD, tag="hs")
                        nc.vector.tensor_copy(out=hs[:], in_=hp[:])
                        nc.sync.dma_start(out=h_shard[t * P:(t + 1) * P, :],
                                          in_=hs[:])
                with nc.named_scope(f"L{layer}_allgather"):
                    if "noB" not in ablate:
                        nc.gpsimd.collective_compute(
                        "AllGather", mybir.AluOpType.bypass,
                            ins=[h_shard[:]], outs=[h_table[:]],
                            replica_groups=groups)
                with nc.named_scope(f"L{layer}_aggregate"):
                    blk0 = 0    # running S^T block offset
                    gmsg0 = 0   # running gathered-message offset
                    for t0 in range(0, cfg.tpc if "noC" not in ablate
                                    else 0, cfg.chunk):
                        nt = min(cfg.chunk, cfg.tpc - t0)
                        nb_chunk = nt * cfg.cpt
                        msg = msgp.tile([P, chunk_cols], _HD, tag="msg")
                        for q in range(N_QUART if "nogather" not in ablate
                                       else 0):
                            nb_q = nt * cfg.qb
                            nidx = nb_q * P
                            c0 = q * nb_q * P
                            i0 = (gmsg0 + q * nidx) // 16
                            nc.gpsimd.dma_gather(
                                out_ap=msg[:, c0:c0 + nidx].rearrange(
                                    "p (b e) -> p b e", e=P),
                                in_ap=h_table[q * cfg.qrows:
                                              (q + 1) * cfg.qrows, :],
                                idxs_ap=idxt[:, i0:i0 + nidx // 16],
                                num_idxs=nidx,
                                num_idxs_reg=nidx,
                                elem_size=P,
                            )
                        # self-loop rows: contiguous read of own h shard
                        sc0 = nt * cfg.gpt * P
                        nc.sync.dma_start(
                            out=msg[:, sc0:sc0 + nt * P].rearrange(
                                "p (b e) -> p b e", e=P),
                            in_=h_shard[t0 * P:(t0 + nt) * P, :].rearrange(
                                "(b p) e -> p b e", p=P))
                        st = stp.tile([P, chunk_cols], _HD, tag="st")
                        if "nost" not in ablate:
                            nc.sync.dma_start(
                                out=st[:, :nb_chunk * P],
                                in_=sts_in[:, blk0 * P:(blk0 + nb_chunk) * P])
                        for ti in range(nt if "nomm" not in ablate else 0):
                            t = t0 + ti
                            op = ops.tile([D, P], _FP, tag="op")
                            ci = 0
                            last = cfg.cpt - 1
                            for q in range(N_QUART):
                                for qb_i in range(cfg.qb):
                                    col = ((q * nt + ti) * cfg.qb
                                           + qb_i) * P
                                    nc.tensor.matmul(
                                        op[:], lhsT=msg[:, col:col + P],
                                        rhs=st[:, col:col + P],
                                        start=(ci == 0),
                                        stop=(ci == last))
                                    ci += 1
                            # self block
                            mcol = sc0 + ti * P
                            scol = (nt * cfg.gpt + ti) * P
                            nc.tensor.matmul(
                                op[:], lhsT=msg[:, mcol:mcol + P],
                                rhs=st[:, scol:scol + P],
                                start=False, stop=True)
                            cs = bass.ts(t, P)
                            if layer < 2:
                                nc.vector.tensor_scalar_add(
                                    out=xnxt[:, cs], in0=op[:],
                                    scalar1=bts[layer][:])
                            else:
                                ob = osbp.tile([D, P], _FP, tag="ob")
                                nc.vector.tensor_scalar_add(
                                    out=ob[:], in0=op[:],
                                    scalar1=bts[layer][:])
                                nc.sync.dma_start(out=out_dram[:, cs],
                                                  in_=ob[:])
                        blk0 += nb_chunk
                        gmsg0 += nt * cfg.gpt * P
    nc.compile()
    return nc


_PROGRAM_CACHE = {}


def _get_program(cfg):
    key = (cfg.n_nodes, cfg.n_edges, cfg.tpc, cfg.qb, cfg.chunk)
    if key not in _PROGRAM_CACHE:
        _PROGRAM_CACHE[key] = build_program(cfg)
    return _PROGRAM_CACHE[key]


# --------------------------------------------------------------- driver


def run(x, edge_index, W1, b1, W2, b2, W3, b3, cfg, trace=False,
        trace_kwargs=None):
    per_core, row_id = prepare(x, edge_index, cfg)
    nc = _get_program(cfg)
    ws = [np.asarray(a, dtype=np.float32) for a in (W1, W2, W3)]
    bs = [np.asarray(a, dtype=np.float32).reshape(D, 1) for a in (b1, b2, b3)]
    in_maps = []
    for k in range(N_CORES):
        m = dict(per_core[k])
        for i in range(3):
            m[f"W{i}"] = ws[i]
            m[f"b{i}"] = bs[i]
        in_maps.append(m)
    res = run_bass_kernel_spmd(nc, in_maps, list(range(N_CORES)),
                               trace=trace, **(trace_kwargs or {}))
    outT = np.concatenate([res.results[k]["out"] for k in range(N_CORES)],
                          axis=1)
    out = np.empty((cfg.n_nodes, D), dtype=np.float32)
    out[:, :] = outT[:, row_id].T
    return out, res


def kernel(x, edge_index, W1, b1, W2, b2, W3, b3):
    out, _ = run(x, edge_index, W1, b1, W2, b2, W3, b3, FULL)
    return out
